# revision 30
# baseline (speedup 1.0000x reference)
"""LSTM layer kernel for Trainium2 (8 NeuronCores, Bass/Tile).

Reference computation (fp32):
    z = concat([x, h], axis=1)                 # [B, IN+OUT]
    f = sigmoid(z @ w_f + b_f)
    i = sigmoid(z @ w_i + b_i)
    g = tanh   (z @ w_c + b_c)
    o = sigmoid(z @ w_o + b_o)
    c_new = c * f + i * g
    h_new = tanh(c_new) * o                    # [B, OUT]

Shapes: B=4096, IN=OUT=1024, K=IN+OUT=2048.

Sharding (8 cores): 2-D grid, 4 batch-groups x 2 output-column-groups.
Core (i, j) computes h_new[i*1024:(i+1)*1024, j*512:(j+1)*512].
Per-core HBM traffic: 8 MiB zT + 16 MiB weights + 2 MiB cT + 2 MiB out
= 28 MiB (vs 40 MiB for pure batch-parallel).  No collectives.

Device layout: everything is transposed so the contraction dim (k) and the
output-channel dim (o) sit on SBUF partitions:
    zT [2048, 1024]  (k, b)       - moving operand (f32r)
    w  [2048, 4, 4, 128] (k, oc, gate, p) - stationary operand, fused across
         gates so one DMA covers all four gates of an o-chunk
    out = w.T @ zT -> psum [o, b], so the per-o bias is a per-partition
    ACT bias and sigmoid/tanh run straight out of PSUM.
The host pre-transposes the x/h/c shards and transposes the per-core
h_newT shards back when assembling the full output.

Matmuls run in float32r (fp32 bits, reduced-precision PE path): 1 cycle/row
at N=512 (same rate as bf16) vs 4 cycles/row for strict fp32.

Startup pipelining: the first o-chunk's weights stream per-ko interleaved
with the z chunks (one fused DMA per ko), so the PE starts matmuls ~3 us in
instead of waiting for the full 8 MiB z load; HWDGE issue-rate (~0.6 us per
dma_start on one ring) is why the weights are gate-fused into few DMAs.
"""

import numpy as np

import concourse.bass as bass
import concourse.tile as tile
from concourse import bacc
from concourse import mybir
from concourse.bass_utils import run_bass_kernel_spmd

P = 128
B_FULL, IN, OUT = 4096, 1024, 1024
K = IN + OUT                 # 2048 contraction
RB, RO = 4, 2                # batch-shards x out-col-shards = 8 cores
B_L = B_FULL // RB           # 1024 batch rows per core
O_L = OUT // RO              # 512 out cols per core
KO = K // P                  # 16 k-chunks
OC = O_L // P                # 4 out chunks per core
NG = 4                       # gates
NT = 512                     # moving free dim per matmul (fp32 max)
NB = B_L // NT               # 2 batch tiles

F32 = mybir.dt.float32
F32R = mybir.dt.float32r
GATES = ("f", "i", "c", "o")

# exec time of the most recent traced run (ns); set by _run when trace=True
last_exec_time_ns = None

_NC_CACHE = {}


def _build_nc(loop_r=None, ko_limit=None):
    # loop_r: timing-only variant that repeats the whole body in a hardware
    # For_i loop, so per-iteration device time can be measured as a wall-clock
    # slope between two loop_r values (the per-exec RPC overhead is ~90 ms,
    # far above the ~134 us kernel).
    # Bacc (not plain Bass): its compile() pipeline runs
    # move_matmul_waits_to_ldweights + generate_event_semaphores, which split
    # multi-wait instructions to satisfy the TRN2 1-wait-per-inst constraint.
    nc = bacc.Bacc()

    zT = nc.dram_tensor("zT", [K, B_L], F32R, kind="ExternalInput")
    cT = nc.dram_tensor("cT", [O_L, B_L], F32, kind="ExternalInput")
    # gate-fused weights: [k, oc, gate, p] with o_local = oc*128 + p
    wA = nc.dram_tensor("wA", [K, OC, NG, P], F32R, kind="ExternalInput")
    # gate-fused biases: [p, oc, gate]
    bA = nc.dram_tensor("bA", [P, OC, NG], F32, kind="ExternalInput")
    hT = nc.dram_tensor("hT", [O_L, B_L], F32, kind="ExternalOutput")

    zT_t = zT[:, :].rearrange("(ko kp) b -> kp ko b", kp=P)    # [128,16,1024]
    cT_t = cT[:, :].rearrange("(oc p) b -> p oc b", p=P)       # [128,4,1024]
    hT_t = hT[:, :].rearrange("(oc p) b -> p oc b", p=P)
    wA_t = wA[:, :, :, :].rearrange(
        "(ko kp) oc g p -> kp ko oc (g p)", kp=P
    )                                                          # [128,16,4,512]

    sig = mybir.ActivationFunctionType.Sigmoid
    tanh = mybir.ActivationFunctionType.Tanh

    import contextlib

    with tile.TileContext(nc) as tc:
        with (
            tc.For_i(0, loop_r, 1) if loop_r else contextlib.nullcontext(),
            tc.tile_pool(name="zpool", bufs=1) as zpool,
            tc.tile_pool(name="cpool", bufs=2) as cpool,
            tc.tile_pool(name="bpool", bufs=1) as bpool,
            tc.tile_pool(name="wpool", bufs=3) as wpool,
            tc.tile_pool(name="gates", bufs=1) as gpool,
            tc.tile_pool(name="temps", bufs=2) as tpool,
            tc.tile_pool(name="psum", bufs=8, space="PSUM") as psum_pool,
        ):
            z_sb = zpool.tile([P, KO, B_L], F32R)          # 8 MiB resident
            w_tiles = [
                wpool.tile([P, KO, NG * P], F32R, tag="w", name=f"w_oc{oc}")
                for oc in range(OC)
            ]
            c_tiles = [
                cpool.tile([P, B_L], F32, tag="c", name=f"c_oc{oc}")
                for oc in range(OC)
            ]

            # DMAs on one HWDGE ring run FIFO, so emission order here IS the
            # arrival order.  Bias goes first (activations gate PSUM reuse on
            # it); then one z chunk + one fused oc0 w slice per ko; the oc0 c
            # chunk rides in the middle so the first combine isn't blocked;
            # oc1's w tile streams per-ko right behind (a whole-tile DMA here
            # costs a ~12 us PE stall); oc2/oc3 prefetch as whole tiles.
            b_sb = bpool.tile([P, OC, NG], F32)
            nc.sync.dma_start(b_sb[:, :, :], bA[:, :, :])
            for ko in range(KO):
                if ko == 0:
                    # halve the first chunk (and load the first w slice in
                    # between) so the first matmul starts sooner
                    nc.sync.dma_start(z_sb[:, 0, 0:NT], zT_t[:, 0, 0:NT])
                    nc.sync.dma_start(w_tiles[0][:, 0, :], wA_t[:, 0, 0, :])
                    nc.sync.dma_start(z_sb[:, 0, NT:B_L], zT_t[:, 0, NT:B_L])
                    continue
                nc.sync.dma_start(z_sb[:, ko, :], zT_t[:, ko, :])
                nc.sync.dma_start(w_tiles[0][:, ko, :], wA_t[:, ko, 0, :])
            nc.sync.dma_start(c_tiles[0][:, :], cT_t[:, 0, :])
            for ko in range(KO):
                nc.sync.dma_start(w_tiles[1][:, ko, :], wA_t[:, ko, 1, :])
                if ko == 8:
                    nc.sync.dma_start(c_tiles[1][:, :], cT_t[:, 1, :])
            nc.sync.dma_start(w_tiles[2][:, :, :], wA_t[:, :, 2, :])
            nc.sync.dma_start(c_tiles[2][:, :], cT_t[:, 2, :])
            nc.sync.dma_start(w_tiles[3][:, :, :], wA_t[:, :, 3, :])
            nc.sync.dma_start(c_tiles[3][:, :], cT_t[:, 3, :])

            for oc in range(OC):
                w_sb = w_tiles[oc]
                c_sb = c_tiles[oc]

                gate_sb = {}
                cf_sb = {}
                for gi, g in enumerate(GATES):
                    ps = [
                        psum_pool.tile([P, NT], F32, tag="ps", name="ps")
                        for _ in range(NB)
                    ]
                    ko_hi = ko_limit or KO
                    for ko in range(ko_hi):
                        for nb in range(NB):
                            nc.tensor.matmul(
                                ps[nb][:, :],
                                lhsT=w_sb[:, ko, gi * P:(gi + 1) * P],
                                rhs=z_sb[:, ko, nb * NT:(nb + 1) * NT],
                                start=(ko == 0),
                                stop=(ko == ko_hi - 1),
                            )
                    func = tanh if g == "c" else sig
                    for nb in range(NB):
                        gt = gpool.tile(
                            [P, NT], F32, tag=f"gate_{g}_{nb}",
                            name=f"gate_{g}_{nb}",
                        )
                        nc.scalar.activation(
                            gt[:, :], ps[nb][:, :], func,
                            bias=b_sb[:, oc, gi:gi + 1],
                        )
                        gate_sb[(g, nb)] = gt

                    if g == "c":
                        # tanh(c*f + i*g) does not depend on gate o — emit it
                        # now so only mul+DMA remain after the last matmul
                        for nb in range(NB):
                            bsl = slice(nb * NT, (nb + 1) * NT)
                            cf = tpool.tile([P, NT], F32, tag="cf",
                                            name=f"cf_{nb}")
                            nc.vector.tensor_mul(
                                cf[:, :], c_sb[:, bsl], gate_sb[("f", nb)][:, :]
                            )
                            ig = tpool.tile([P, NT], F32, tag="ig", name="ig")
                            nc.vector.tensor_mul(
                                ig[:, :], gate_sb[("i", nb)][:, :],
                                gate_sb[("c", nb)][:, :],
                            )
                            nc.vector.tensor_add(cf[:, :], cf[:, :], ig[:, :])
                            nc.scalar.activation(cf[:, :], cf[:, :], tanh)
                            cf_sb[nb] = cf

                for nb in range(NB):
                    bsl = slice(nb * NT, (nb + 1) * NT)
                    cf = cf_sb[nb]
                    nc.vector.tensor_mul(
                        cf[:, :], cf[:, :], gate_sb[("o", nb)][:, :]
                    )
                    nc.sync.dma_start(hT_t[:, oc, bsl], cf[:, :])

    # run the Bacc pass pipeline (alloc_regs, wait-splitting, ...);
    # run_bass_via_pjrt does not finalize on our behalf
    nc.finalize()
    return nc


def _get_nc():
    if "nc" not in _NC_CACHE:
        _NC_CACHE["nc"] = _build_nc()
    return _NC_CACHE["nc"]


def _shard_inputs(x, h, c, w_f, b_f, w_i, b_i, w_c, b_c, w_o, b_o):
    ws = {"f": w_f, "i": w_i, "c": w_c, "o": w_o}
    bz = {"f": b_f, "i": b_i, "c": b_c, "o": b_o}
    f32 = np.float32

    # per-out-group fused weight/bias shards (shared by the 4 batch groups)
    # wA[k, oc, g, p] = w_g[k, j*O_L + oc*P + p]
    wA_sh = {}
    bA_sh = {}
    for j in range(RO):
        cols = slice(j * O_L, (j + 1) * O_L)
        wA_sh[j] = np.ascontiguousarray(
            np.stack(
                [np.asarray(ws[g][:, cols], dtype=f32).reshape(K, OC, P)
                 for g in GATES],
                axis=2,
            )
        )
        bA_sh[j] = np.ascontiguousarray(
            np.stack(
                [np.asarray(bz[g], dtype=f32).reshape(-1)[cols].reshape(OC, P).T
                 for g in GATES],
                axis=2,
            )
        )

    in_maps = []
    for i in range(RB):
        rows = slice(i * B_L, (i + 1) * B_L)
        zT = np.ascontiguousarray(
            np.concatenate([x[rows], h[rows]], axis=1).T, dtype=f32
        )
        for j in range(RO):
            cT = np.ascontiguousarray(
                c[rows, j * O_L:(j + 1) * O_L].T, dtype=f32
            )
            in_maps.append(
                {"zT": zT, "cT": cT, "wA": wA_sh[j], "bA": bA_sh[j]}
            )
    return in_maps


def _run(in_maps, trace=False, trace_cores=None):
    global last_exec_time_ns
    nc = _get_nc()
    res = run_bass_kernel_spmd(
        nc, in_maps, list(range(RB * RO)),
        trace=trace, trace_cores=trace_cores,
    )
    if trace:
        last_exec_time_ns = res.exec_time_ns
    return res.results


def kernel(x, h, c, w_f, b_f, w_i, b_i, w_c, b_c, w_o, b_o):
    in_maps = _shard_inputs(
        x, h, c, w_f, b_f, w_i, b_i, w_c, b_c, w_o, b_o
    )
    results = _run(in_maps)
    out = np.empty((B_FULL, OUT), np.float32)
    for i in range(RB):
        for j in range(RO):
            shard = results[i * RO + j]["hT"]  # [O_L, B_L]
            out[i * B_L:(i + 1) * B_L, j * O_L:(j + 1) * O_L] = shard.T
    return out


# revision 33
# speedup vs baseline: 1.0028x; 1.0028x over previous
"""LSTM layer kernel for Trainium2 (8 NeuronCores, Bass/Tile).

Reference computation (fp32):
    z = concat([x, h], axis=1)                 # [B, IN+OUT]
    f = sigmoid(z @ w_f + b_f)
    i = sigmoid(z @ w_i + b_i)
    g = tanh   (z @ w_c + b_c)
    o = sigmoid(z @ w_o + b_o)
    c_new = c * f + i * g
    h_new = tanh(c_new) * o                    # [B, OUT]

Shapes: B=4096, IN=OUT=1024, K=IN+OUT=2048.

Sharding (8 cores): 2-D grid, 4 batch-groups x 2 output-column-groups.
Core (i, j) computes h_new[i*1024:(i+1)*1024, j*512:(j+1)*512].
Per-core HBM traffic: 8 MiB zT + 16 MiB weights + 2 MiB cT + 2 MiB out
= 28 MiB (vs 40 MiB for pure batch-parallel).  No collectives.

Device layout: everything is transposed so the contraction dim (k) and the
output-channel dim (o) sit on SBUF partitions:
    zT [2048, 1024]  (k, b)       - moving operand (f32r)
    w  [2048, 4, 4, 128] (k, oc, gate, p) - stationary operand, fused across
         gates so one DMA covers all four gates of an o-chunk
    out = w.T @ zT -> psum [o, b], so the per-o bias is a per-partition
    ACT bias and sigmoid/tanh run straight out of PSUM.
The host pre-transposes the x/h/c shards and transposes the per-core
h_newT shards back when assembling the full output.

Matmuls run in float32r (fp32 bits, reduced-precision PE path): 1 cycle/row
at N=512 (same rate as bf16) vs 4 cycles/row for strict fp32.

Startup pipelining: the first o-chunk's weights stream per-ko interleaved
with the z chunks (one fused DMA per ko), so the PE starts matmuls ~3 us in
instead of waiting for the full 8 MiB z load; HWDGE issue-rate (~0.6 us per
dma_start on one ring) is why the weights are gate-fused into few DMAs.
"""

import numpy as np

import concourse.bass as bass
import concourse.tile as tile
from concourse import bacc
from concourse import mybir
from concourse.bass_utils import run_bass_kernel_spmd

P = 128
B_FULL, IN, OUT = 4096, 1024, 1024
K = IN + OUT                 # 2048 contraction
RB, RO = 4, 2                # batch-shards x out-col-shards = 8 cores
B_L = B_FULL // RB           # 1024 batch rows per core
O_L = OUT // RO              # 512 out cols per core
KO = K // P                  # 16 k-chunks
OC = O_L // P                # 4 out chunks per core
NG = 4                       # gates
NT = 512                     # moving free dim per matmul (fp32 max)
NB = B_L // NT               # 2 batch tiles

F32 = mybir.dt.float32
F32R = mybir.dt.float32r
GATES = ("f", "i", "c", "o")

# exec time of the most recent traced run (ns); set by _run when trace=True
last_exec_time_ns = None

_NC_CACHE = {}


def _build_nc(loop_r=None, ko_limit=None):
    # loop_r: timing-only variant that repeats the whole body in a hardware
    # For_i loop, so per-iteration device time can be measured as a wall-clock
    # slope between two loop_r values (the per-exec RPC overhead is ~90 ms,
    # far above the ~134 us kernel).
    # Bacc (not plain Bass): its compile() pipeline runs
    # move_matmul_waits_to_ldweights + generate_event_semaphores, which split
    # multi-wait instructions to satisfy the TRN2 1-wait-per-inst constraint.
    nc = bacc.Bacc()

    zT = nc.dram_tensor("zT", [K, B_L], F32R, kind="ExternalInput")
    cT = nc.dram_tensor("cT", [O_L, B_L], F32, kind="ExternalInput")
    # gate-fused weights: [k, oc, gate, p] with o_local = oc*128 + p
    wA = nc.dram_tensor("wA", [K, OC, NG, P], F32R, kind="ExternalInput")
    # gate-fused biases: [p, oc, gate]
    bA = nc.dram_tensor("bA", [P, OC, NG], F32, kind="ExternalInput")
    hT = nc.dram_tensor("hT", [O_L, B_L], F32, kind="ExternalOutput")

    zT_t = zT[:, :].rearrange("(ko kp) b -> kp ko b", kp=P)    # [128,16,1024]
    cT_t = cT[:, :].rearrange("(oc p) b -> p oc b", p=P)       # [128,4,1024]
    hT_t = hT[:, :].rearrange("(oc p) b -> p oc b", p=P)
    wA_t = wA[:, :, :, :].rearrange(
        "(ko kp) oc g p -> kp ko oc (g p)", kp=P
    )                                                          # [128,16,4,512]

    sig = mybir.ActivationFunctionType.Sigmoid
    tanh = mybir.ActivationFunctionType.Tanh

    import contextlib

    with tile.TileContext(nc) as tc:
        with (
            tc.For_i(0, loop_r, 1) if loop_r else contextlib.nullcontext(),
            tc.tile_pool(name="zpool", bufs=1) as zpool,
            tc.tile_pool(name="cpool", bufs=2) as cpool,
            tc.tile_pool(name="bpool", bufs=1) as bpool,
            tc.tile_pool(name="wpool", bufs=3) as wpool,
            tc.tile_pool(name="gates", bufs=1) as gpool,
            tc.tile_pool(name="temps", bufs=2) as tpool,
            tc.tile_pool(name="psum", bufs=8, space="PSUM") as psum_pool,
        ):
            z_sb = zpool.tile([P, KO, B_L], F32R)          # 8 MiB resident
            w_tiles = [
                wpool.tile([P, KO, NG * P], F32R, tag="w", name=f"w_oc{oc}")
                for oc in range(OC)
            ]
            c_tiles = [
                cpool.tile([P, B_L], F32, tag="c", name=f"c_oc{oc}")
                for oc in range(OC)
            ]

            # DMAs on one HWDGE ring run FIFO, so emission order here IS the
            # arrival order.  Bias goes first (activations gate PSUM reuse on
            # it); then one z chunk + one fused oc0 w slice per ko; the oc0 c
            # chunk rides in the middle so the first combine isn't blocked;
            # oc1's w tile streams per-ko right behind (a whole-tile DMA here
            # costs a ~12 us PE stall); oc2/oc3 prefetch as whole tiles.
            b_sb = bpool.tile([P, OC, NG], F32)
            nc.scalar.dma_start(b_sb[:, :, :], bA[:, :, :])
            for ko in range(KO):
                if ko == 0:
                    # halve the first chunk (and load the first w slice in
                    # between) so the first matmul starts sooner
                    nc.sync.dma_start(z_sb[:, 0, 0:NT], zT_t[:, 0, 0:NT])
                    nc.sync.dma_start(w_tiles[0][:, 0, :], wA_t[:, 0, 0, :])
                    nc.sync.dma_start(z_sb[:, 0, NT:B_L], zT_t[:, 0, NT:B_L])
                    continue
                nc.sync.dma_start(z_sb[:, ko, :], zT_t[:, ko, :])
                nc.sync.dma_start(w_tiles[0][:, ko, :], wA_t[:, ko, 0, :])
            nc.sync.dma_start(c_tiles[0][:, :], cT_t[:, 0, :])
            for ko in range(KO):
                nc.sync.dma_start(w_tiles[1][:, ko, :], wA_t[:, ko, 1, :])
                if ko == 8:
                    nc.sync.dma_start(c_tiles[1][:, :], cT_t[:, 1, :])
            nc.sync.dma_start(w_tiles[2][:, :, :], wA_t[:, :, 2, :])
            nc.sync.dma_start(c_tiles[2][:, :], cT_t[:, 2, :])
            nc.sync.dma_start(w_tiles[3][:, :, :], wA_t[:, :, 3, :])
            nc.sync.dma_start(c_tiles[3][:, :], cT_t[:, 3, :])

            for oc in range(OC):
                w_sb = w_tiles[oc]
                c_sb = c_tiles[oc]

                gate_sb = {}
                cf_sb = {}
                for gi, g in enumerate(GATES):
                    ps = [
                        psum_pool.tile([P, NT], F32, tag="ps", name="ps")
                        for _ in range(NB)
                    ]
                    ko_hi = ko_limit or KO
                    for ko in range(ko_hi):
                        for nb in range(NB):
                            nc.tensor.matmul(
                                ps[nb][:, :],
                                lhsT=w_sb[:, ko, gi * P:(gi + 1) * P],
                                rhs=z_sb[:, ko, nb * NT:(nb + 1) * NT],
                                start=(ko == 0),
                                stop=(ko == ko_hi - 1),
                            )
                    func = tanh if g == "c" else sig
                    for nb in range(NB):
                        gt = gpool.tile(
                            [P, NT], F32, tag=f"gate_{g}_{nb}",
                            name=f"gate_{g}_{nb}",
                        )
                        nc.scalar.activation(
                            gt[:, :], ps[nb][:, :], func,
                            bias=b_sb[:, oc, gi:gi + 1],
                        )
                        gate_sb[(g, nb)] = gt

                    if g == "c":
                        # tanh(c*f + i*g) does not depend on gate o — emit it
                        # now so only mul+DMA remain after the last matmul
                        for nb in range(NB):
                            bsl = slice(nb * NT, (nb + 1) * NT)
                            cf = tpool.tile([P, NT], F32, tag="cf",
                                            name=f"cf_{nb}")
                            nc.vector.tensor_mul(
                                cf[:, :], c_sb[:, bsl], gate_sb[("f", nb)][:, :]
                            )
                            ig = tpool.tile([P, NT], F32, tag="ig", name="ig")
                            nc.vector.tensor_mul(
                                ig[:, :], gate_sb[("i", nb)][:, :],
                                gate_sb[("c", nb)][:, :],
                            )
                            nc.vector.tensor_add(cf[:, :], cf[:, :], ig[:, :])
                            nc.scalar.activation(cf[:, :], cf[:, :], tanh)
                            cf_sb[nb] = cf

                for nb in range(NB):
                    bsl = slice(nb * NT, (nb + 1) * NT)
                    cf = cf_sb[nb]
                    nc.vector.tensor_mul(
                        cf[:, :], cf[:, :], gate_sb[("o", nb)][:, :]
                    )
                    nc.sync.dma_start(hT_t[:, oc, bsl], cf[:, :])

    # run the Bacc pass pipeline (alloc_regs, wait-splitting, ...);
    # run_bass_via_pjrt does not finalize on our behalf
    nc.finalize()
    return nc


def _get_nc():
    if "nc" not in _NC_CACHE:
        _NC_CACHE["nc"] = _build_nc()
    return _NC_CACHE["nc"]


def _shard_inputs(x, h, c, w_f, b_f, w_i, b_i, w_c, b_c, w_o, b_o):
    ws = {"f": w_f, "i": w_i, "c": w_c, "o": w_o}
    bz = {"f": b_f, "i": b_i, "c": b_c, "o": b_o}
    f32 = np.float32

    # per-out-group fused weight/bias shards (shared by the 4 batch groups)
    # wA[k, oc, g, p] = w_g[k, j*O_L + oc*P + p]
    wA_sh = {}
    bA_sh = {}
    for j in range(RO):
        cols = slice(j * O_L, (j + 1) * O_L)
        wA_sh[j] = np.ascontiguousarray(
            np.stack(
                [np.asarray(ws[g][:, cols], dtype=f32).reshape(K, OC, P)
                 for g in GATES],
                axis=2,
            )
        )
        bA_sh[j] = np.ascontiguousarray(
            np.stack(
                [np.asarray(bz[g], dtype=f32).reshape(-1)[cols].reshape(OC, P).T
                 for g in GATES],
                axis=2,
            )
        )

    in_maps = []
    for i in range(RB):
        rows = slice(i * B_L, (i + 1) * B_L)
        zT = np.ascontiguousarray(
            np.concatenate([x[rows], h[rows]], axis=1).T, dtype=f32
        )
        for j in range(RO):
            cT = np.ascontiguousarray(
                c[rows, j * O_L:(j + 1) * O_L].T, dtype=f32
            )
            in_maps.append(
                {"zT": zT, "cT": cT, "wA": wA_sh[j], "bA": bA_sh[j]}
            )
    return in_maps


def _run(in_maps, trace=False, trace_cores=None):
    global last_exec_time_ns
    nc = _get_nc()
    res = run_bass_kernel_spmd(
        nc, in_maps, list(range(RB * RO)),
        trace=trace, trace_cores=trace_cores,
    )
    if trace:
        last_exec_time_ns = res.exec_time_ns
    return res.results


def kernel(x, h, c, w_f, b_f, w_i, b_i, w_c, b_c, w_o, b_o):
    in_maps = _shard_inputs(
        x, h, c, w_f, b_f, w_i, b_i, w_c, b_c, w_o, b_o
    )
    results = _run(in_maps)
    out = np.empty((B_FULL, OUT), np.float32)
    for i in range(RB):
        for j in range(RO):
            shard = results[i * RO + j]["hT"]  # [O_L, B_L]
            out[i * B_L:(i + 1) * B_L, j * O_L:(j + 1) * O_L] = shard.T
    return out
